# revision 5
# baseline (speedup 1.0000x reference)
"""Self-contained Trainium2 kernel for nn_DynamicConv2D (moe_routing).

Contract: kernel(**inputs) takes FULL unsharded inputs (numpy), returns the
FULL output [32, 64, 64, 128] float32. Internally shards batch across 8
NeuronCores (4 samples each), runs a Bass/Tile kernel via
run_bass_kernel_spmd, and gathers.

Device-side work per sample:
  pool  = sum(x) over H,W            (DVE free-dim reduce; 1/4096 folded into R)
  att   = softmax(relu(pool@R')@A')  (tiny PE matmuls + ACT relu/exp + DVE recip)
  wmix  = sum_k att[k] * bank[k]     (4 fused DVE scalar_tensor_tensor MACs)
  conv  = 9-tap shifted matmuls (f32r) accumulated in PSUM, per 512-pos chunk
  out   = Relu(conv + beta)          (ACT epilogue, per-partition bias;
                                      BN scale folded into bank/bias on host)

Layout: x is host-transposed to channel-major [C, H, W] and zero-padded to
[C, 66, 66] so all 9 conv taps are plain access-pattern offsets; output is
produced channel-major [F, H*W] and host-transposed back to NHWC.
"""

import sys

if "/opt/trn_rl_repo" not in sys.path:
    sys.path.insert(0, "/opt/trn_rl_repo")

import numpy as np

import concourse.bacc as bacc
import concourse.tile as tile
from concourse import mybir
from concourse.bass_utils import run_bass_kernel_spmd

def _ensure_ntff_hook():
    """run_bass_kernel_spmd(trace=True) under axon needs antenv.axon_hooks,
    which this image's antenv package lacks. Register an equivalent module
    (ctypes into libaxon_pjrt.so) so profiled runs work."""
    try:
        from antenv import axon_hooks  # noqa: F401
        return
    except ImportError:
        pass
    import contextlib
    import ctypes
    import os
    import types

    so_path = os.environ.get("AXON_PJRT_SO", "/opt/axon/libaxon_pjrt.so")
    mod = types.ModuleType("antenv.axon_hooks")
    state = {"hook": None}

    def _make_hook():
        if not os.path.exists(so_path):
            return None
        lib = ctypes.CDLL(so_path)
        if not hasattr(lib, "axon_start_nrt_profile"):
            return None
        lib.axon_start_nrt_profile.argtypes = [
            ctypes.POINTER(ctypes.c_int64), ctypes.c_size_t]
        lib.axon_start_nrt_profile.restype = ctypes.c_int64
        lib.axon_stop_nrt_profile.argtypes = [ctypes.c_char_p]
        lib.axon_stop_nrt_profile.restype = ctypes.c_int64

        @contextlib.contextmanager
        def _hook(output_dir, device_ids):
            import jax
            jax.devices()
            if device_ids:
                ids = (ctypes.c_int64 * len(device_ids))(*device_ids)
                rc = lib.axon_start_nrt_profile(ids, len(device_ids))
            else:
                rc = lib.axon_start_nrt_profile(None, 0)
            if rc != 0:
                raise RuntimeError(f"axon_start_nrt_profile rc={rc}")
            try:
                yield
            finally:
                n = lib.axon_stop_nrt_profile(str(output_dir).encode())
                if n < 0:
                    raise RuntimeError(f"axon_stop_nrt_profile rc={n}")

        return _hook

    def get_axon_ntff_profile_hook():
        if state["hook"] is None:
            state["hook"] = _make_hook()
        return state["hook"]

    def set_axon_ntff_profile_hook(hook):
        state["hook"] = hook

    mod.get_axon_ntff_profile_hook = get_axon_ntff_profile_hook
    mod.set_axon_ntff_profile_hook = set_axon_ntff_profile_hook
    sys.modules["antenv.axon_hooks"] = mod
    try:
        import antenv
        antenv.axon_hooks = mod
    except ImportError:
        pass


F32 = mybir.dt.float32
F32R = mybir.dt.float32r
AF = mybir.ActivationFunctionType
ALU = mybir.AluOpType

B, H, W, C = 32, 64, 64, 128
NCORES = 8
BPC = B // NCORES  # samples per core
HP, WP = H + 2, W + 2  # zero-padded
NPAD = HP * WP  # 4356
NPOS = H * W  # 4096
K = 4  # experts
NF = 128  # output filters
TAPS = 9
ROWS_PER_CHUNK = 8  # 8 image rows * 64 cols = 512 positions per PSUM chunk
NCHUNK = H // ROWS_PER_CHUNK


def _emit_chain(nc, b, sb, ps, xt_sb, consts, wk_sb, wm_sb, beta_sb):
    """Routing chain for sample b: pool -> attention -> mixed weights + beta."""
    red_sb, attk_sb, i4_sb, ones1_sb, ones4_sb, biasw_sb, c1_sb = consts

    # Global pool: sum over all padded positions (zeros don't change the sum).
    pool_sb = sb.tile([C, 1], F32, tag="pool")
    nc.vector.tensor_reduce(pool_sb[:], xt_sb[:], axis=mybir.AxisListType.X,
                            op=ALU.add)

    # pool_red.T = (R/4096).T @ pool_sum  -> [4, 1]
    pr_ps = ps.tile([K, 1], F32, tag="tiny")
    nc.tensor.matmul(pr_ps[:], red_sb[:], pool_sb[:], start=True, stop=True)
    prelu_sb = sb.tile([K, 1], F32, tag="prelu")
    nc.scalar.activation(prelu_sb[:], pr_ps[:], AF.Relu)

    # logits.T = (A/30).T @ pool_red.T  -> [4, 1]
    lg_ps = ps.tile([K, 1], F32, tag="tiny")
    nc.tensor.matmul(lg_ps[:], attk_sb[:], prelu_sb[:], start=True, stop=True)
    e_sb = sb.tile([K, 1], F32, tag="esb")
    nc.scalar.activation(e_sb[:], lg_ps[:], AF.Exp)

    # softmax denominator and broadcast of its reciprocal to 4 partitions
    s_ps = ps.tile([1, 1], F32, tag="tiny")
    nc.tensor.matmul(s_ps[:], ones4_sb[:], e_sb[:], start=True, stop=True)
    rec_sb = sb.tile([1, 1], F32, tag="rec")
    nc.vector.reciprocal(rec_sb[:], s_ps[:])
    r4_ps = ps.tile([K, 1], F32, tag="tiny")
    nc.tensor.matmul(r4_ps[:], ones1_sb[:, 0:K], rec_sb[:], start=True,
                     stop=True)
    att_sb = sb.tile([K, 1], F32, tag="att")
    nc.vector.tensor_mul(att_sb[:], e_sb[:], r4_ps[:])

    # beta = biasw.T @ att + c1   (bias and BN folded; per-partition over F)
    bm_ps = ps.tile([NF, 1], F32, tag="tiny")
    nc.tensor.matmul(bm_ps[:], biasw_sb[:], att_sb[:], start=True, stop=True)
    nc.scalar.activation(beta_sb[:], bm_ps[:], AF.Identity, bias=c1_sb[:])

    # Broadcast att to all 128 partitions: att_row = att.T @ I4 -> [1, 4],
    # then att_bc = ones(1,128).T @ att_row -> [128, 4].
    ar_ps = ps.tile([1, K], F32, tag="tiny")
    nc.tensor.matmul(ar_ps[:], att_sb[:], i4_sb[:], start=True, stop=True)
    ar_sb = sb.tile([1, K], F32, tag="arow")
    nc.scalar.copy(ar_sb[:], ar_ps[:])
    ab_ps = ps.tile([C, K], F32, tag="tiny")
    nc.tensor.matmul(ab_ps[:], ones1_sb[:], ar_sb[:], start=True, stop=True)
    ab_sb = sb.tile([C, K], F32, tag="abc")
    nc.scalar.copy(ab_sb[:], ab_ps[:])

    # Mix expert bank: wm = sum_k att[k] * wk[k]   [128, 1152] -> f32r
    nc.vector.tensor_scalar_mul(wm_sb[:], wk_sb[0][:], ab_sb[:, 0:1])
    for k in range(1, K):
        nc.vector.scalar_tensor_tensor(wm_sb[:], wk_sb[k][:], ab_sb[:, k:k + 1],
                                       wm_sb[:], op0=ALU.mult, op1=ALU.add)


def _emit_conv(nc, b, sb, convps, xt_sb, wm_sb, beta_sb, y_dram):
    """9-tap conv as shifted f32r matmuls + fused BN/bias/relu epilogue."""
    xv = xt_sb[:].rearrange("p (h w) -> p h w", w=WP)
    y_sb = sb.tile([NF, NPOS], F32, tag="ystage")
    for t in range(NCHUNK):
        pc = convps.tile([NF, ROWS_PER_CHUNK * W], F32, tag="conv")
        for tap in range(TAPS):
            dy, dx = tap // 3, tap % 3
            rhs = xv[:, ROWS_PER_CHUNK * t + dy:ROWS_PER_CHUNK * t + dy + ROWS_PER_CHUNK,
                     dx:dx + W]
            nc.tensor.matmul(pc[:], wm_sb[:, NF * tap:NF * (tap + 1)], rhs,
                             start=(tap == 0), stop=(tap == TAPS - 1))
        nc.scalar.activation(y_sb[:, 512 * t:512 * (t + 1)], pc[:], AF.Relu,
                             bias=beta_sb[:])
    nc.sync.dma_start(y_dram[b], y_sb[:])


def _build_program():
    nc = bacc.Bacc("TRN2", target_bir_lowering=False, debug=False,
                   num_devices=NCORES)
    xt = nc.dram_tensor("xt", [BPC, C, NPAD], F32R, kind="ExternalInput").ap()
    wk = nc.dram_tensor("wk", [K, C, TAPS * NF], F32, kind="ExternalInput").ap()
    biasw = nc.dram_tensor("biasw", [K, NF], F32, kind="ExternalInput").ap()
    c1 = nc.dram_tensor("c1", [NF, 1], F32, kind="ExternalInput").ap()
    red = nc.dram_tensor("red", [C, K], F32, kind="ExternalInput").ap()
    attk = nc.dram_tensor("attk", [K, K], F32, kind="ExternalInput").ap()
    i4 = nc.dram_tensor("i4", [K, K], F32, kind="ExternalInput").ap()
    ones1 = nc.dram_tensor("ones1", [1, C], F32, kind="ExternalInput").ap()
    ones4 = nc.dram_tensor("ones4", [K, 1], F32, kind="ExternalInput").ap()
    y = nc.dram_tensor("y", [BPC, NF, NPOS], F32, kind="ExternalOutput").ap()

    with tile.TileContext(nc) as tc:
        with (
            tc.tile_pool(name="const", bufs=1) as cpool,
            tc.tile_pool(name="xt", bufs=BPC) as xpool,
            tc.tile_pool(name="wm", bufs=BPC) as wmpool,
            tc.tile_pool(name="work", bufs=4) as sb,
            tc.tile_pool(name="ystage", bufs=2) as ypool,
            tc.tile_pool(name="convps", bufs=4, space="PSUM") as convps,
            tc.tile_pool(name="tinyps", bufs=3, space="PSUM") as ps,
        ):
            # Constants
            red_sb = cpool.tile([C, K], F32)
            attk_sb = cpool.tile([K, K], F32)
            i4_sb = cpool.tile([K, K], F32)
            ones1_sb = cpool.tile([1, C], F32)
            ones4_sb = cpool.tile([K, 1], F32)
            biasw_sb = cpool.tile([K, NF], F32)
            c1_sb = cpool.tile([NF, 1], F32)
            nc.sync.dma_start(red_sb[:], red)
            nc.sync.dma_start(attk_sb[:], attk)
            nc.sync.dma_start(i4_sb[:], i4)
            nc.sync.dma_start(ones1_sb[:], ones1)
            nc.sync.dma_start(ones4_sb[:], ones4)
            nc.sync.dma_start(biasw_sb[:], biasw)
            nc.sync.dma_start(c1_sb[:], c1)
            wk_sb = []
            for k in range(K):
                wkt = cpool.tile([C, TAPS * NF], F32, tag=f"wk{k}")
                nc.sync.dma_start(wkt[:], wk[k])
                wk_sb.append(wkt)
            consts = (red_sb, attk_sb, i4_sb, ones1_sb, ones4_sb, biasw_sb,
                      c1_sb)

            # Pre-load the ACT spline table set (relu+exp share one set).
            warm_sb = cpool.tile([1, 1], F32, tag="warm")
            nc.scalar.activation(warm_sb[:], ones4_sb[0:1, :], AF.Exp)

            # Stage all four samples' inputs up front.
            xt_sb = []
            for b in range(BPC):
                xtt = xpool.tile([C, NPAD], F32R, tag="xt")
                nc.sync.dma_start(xtt[:], xt[b])
                xt_sb.append(xtt)

            wm_sb = [wmpool.tile([C, TAPS * NF], F32R, tag="wm",
                                 name=f"wm{b}") for b in range(BPC)]
            beta_sb = [sb.tile([NF, 1], F32, tag="beta", name=f"beta{b}")
                       for b in range(BPC)]

            # chain(0), chain(1), conv(0), chain(2), conv(1), ... keeps the
            # next sample's routing ahead of the PE conv stream.
            _emit_chain(nc, 0, sb, ps, xt_sb[0], consts, wk_sb, wm_sb[0],
                        beta_sb[0])
            for b in range(BPC):
                if b + 1 < BPC:
                    _emit_chain(nc, b + 1, sb, ps, xt_sb[b + 1], consts, wk_sb,
                                wm_sb[b + 1], beta_sb[b + 1])
                _emit_conv(nc, b, ypool, convps, xt_sb[b], wm_sb[b],
                           beta_sb[b], y)

    nc.compile()
    return nc


_PROGRAM = None


def _get_program():
    global _PROGRAM
    if _PROGRAM is None:
        _PROGRAM = _build_program()
    return _PROGRAM


def _prepare_host_inputs(x, reduction_kernel, attention_kernel, conv_kernels,
                         bias, bn_scale, bn_bias, bn_mean, bn_var):
    f = np.float32
    # Channel-major zero-padded input: [B, C, 66, 66]
    xt = np.zeros((B, C, HP, WP), dtype=f)
    xt[:, :, 1:H + 1, 1:W + 1] = np.ascontiguousarray(x.transpose(0, 3, 1, 2))
    xt = xt.reshape(B, C, NPAD)

    inv = (bn_scale / np.sqrt(bn_var + np.float32(1e-5))).astype(f)
    # Expert bank [k, C, tap*F], BN scale folded into F.
    wkh = (conv_kernels.transpose(0, 3, 1, 2, 4) * inv).astype(f)
    wkh = np.ascontiguousarray(wkh.reshape(K, C, TAPS * NF))
    biasw = np.ascontiguousarray((bias * inv).astype(f))
    c1 = np.ascontiguousarray((bn_bias - bn_mean * inv).astype(f).reshape(NF, 1))
    red = np.ascontiguousarray((reduction_kernel / np.float32(NPOS)).astype(f))
    attk = np.ascontiguousarray((attention_kernel / np.float32(30.0)).astype(f))
    i4 = np.eye(K, dtype=f)
    ones1 = np.ones((1, C), dtype=f)
    ones4 = np.ones((K, 1), dtype=f)

    shared = {"wk": wkh, "biasw": biasw, "c1": c1, "red": red, "attk": attk,
              "i4": i4, "ones1": ones1, "ones4": ones4}
    in_maps = []
    for cix in range(NCORES):
        m = dict(shared)
        m["xt"] = np.ascontiguousarray(xt[cix * BPC:(cix + 1) * BPC])
        in_maps.append(m)
    return in_maps


def kernel(x, reduction_kernel, attention_kernel, conv_kernels, bias, bn_scale,
           bn_bias, bn_mean, bn_var, _trace=False):
    nc = _get_program()
    in_maps = _prepare_host_inputs(
        np.asarray(x, dtype=np.float32), np.asarray(reduction_kernel, np.float32),
        np.asarray(attention_kernel, np.float32),
        np.asarray(conv_kernels, np.float32), np.asarray(bias, np.float32),
        np.asarray(bn_scale, np.float32), np.asarray(bn_bias, np.float32),
        np.asarray(bn_mean, np.float32), np.asarray(bn_var, np.float32))
    if _trace:
        _ensure_ntff_hook()
    res = run_bass_kernel_spmd(nc, in_maps, core_ids=list(range(NCORES)),
                               trace=_trace)
    yt = np.concatenate([res.results[cix]["y"] for cix in range(NCORES)],
                        axis=0)  # [B, F, 4096]
    out = yt.reshape(B, NF, H, W).transpose(0, 2, 3, 1)
    out = np.ascontiguousarray(out, dtype=np.float32)
    if _trace:
        return out, res
    return out


# revision 21
# speedup vs baseline: 1.0601x; 1.0601x over previous
"""Self-contained Trainium2 kernel for nn_DynamicConv2D (moe_routing).

Contract: kernel(**inputs) takes FULL unsharded inputs (numpy), returns the
FULL output [32, 64, 64, 128] float32. Internally shards batch across 8
NeuronCores (4 samples each), runs a Bass/Tile kernel via
run_bass_kernel_spmd, and gathers.

Device-side work per sample:
  pool  = sum(x) over H,W            (DVE free-dim reduce; 1/4096 folded into R)
  att   = softmax(relu(pool@R')@A')  (tiny PE matmuls + ACT relu/exp + DVE recip)
  wmix  = sum_k att[k] * bank[k]     (4 fused DVE scalar_tensor_tensor MACs)
  conv  = 9-tap shifted matmuls (f32r) accumulated in PSUM, per 512-pos chunk
  out   = Relu(conv + beta)          (ACT epilogue, per-partition bias;
                                      BN scale folded into bank/bias on host)

Layout: x is host-transposed to channel-major [C, H, W] and zero-padded to
[C, 66, 66] so all 9 conv taps are plain access-pattern offsets; output is
produced channel-major [F, H*W] and host-transposed back to NHWC.
"""

import sys

if "/opt/trn_rl_repo" not in sys.path:
    sys.path.insert(0, "/opt/trn_rl_repo")

import numpy as np

import concourse.bacc as bacc
import concourse.tile as tile
from concourse import mybir
from concourse.bass_utils import run_bass_kernel_spmd
from concourse.tile_rust import add_dep_helper

def _ensure_ntff_hook():
    """run_bass_kernel_spmd(trace=True) under axon needs antenv.axon_hooks,
    which this image's antenv package lacks. Register an equivalent module
    (ctypes into libaxon_pjrt.so) so profiled runs work."""
    try:
        from antenv import axon_hooks  # noqa: F401
        return
    except ImportError:
        pass
    import contextlib
    import ctypes
    import os
    import types

    so_path = os.environ.get("AXON_PJRT_SO", "/opt/axon/libaxon_pjrt.so")
    mod = types.ModuleType("antenv.axon_hooks")
    state = {"hook": None}

    def _make_hook():
        if not os.path.exists(so_path):
            return None
        lib = ctypes.CDLL(so_path)
        if not hasattr(lib, "axon_start_nrt_profile"):
            return None
        lib.axon_start_nrt_profile.argtypes = [
            ctypes.POINTER(ctypes.c_int64), ctypes.c_size_t]
        lib.axon_start_nrt_profile.restype = ctypes.c_int64
        lib.axon_stop_nrt_profile.argtypes = [ctypes.c_char_p]
        lib.axon_stop_nrt_profile.restype = ctypes.c_int64

        @contextlib.contextmanager
        def _hook(output_dir, device_ids):
            import jax
            jax.devices()
            if device_ids:
                ids = (ctypes.c_int64 * len(device_ids))(*device_ids)
                rc = lib.axon_start_nrt_profile(ids, len(device_ids))
            else:
                rc = lib.axon_start_nrt_profile(None, 0)
            if rc != 0:
                raise RuntimeError(f"axon_start_nrt_profile rc={rc}")
            try:
                yield
            finally:
                n = lib.axon_stop_nrt_profile(str(output_dir).encode())
                if n < 0:
                    raise RuntimeError(f"axon_stop_nrt_profile rc={n}")

        return _hook

    def get_axon_ntff_profile_hook():
        if state["hook"] is None:
            state["hook"] = _make_hook()
        return state["hook"]

    def set_axon_ntff_profile_hook(hook):
        state["hook"] = hook

    mod.get_axon_ntff_profile_hook = get_axon_ntff_profile_hook
    mod.set_axon_ntff_profile_hook = set_axon_ntff_profile_hook
    sys.modules["antenv.axon_hooks"] = mod
    try:
        import antenv
        antenv.axon_hooks = mod
    except ImportError:
        pass


F32 = mybir.dt.float32
F32R = mybir.dt.float32r
AF = mybir.ActivationFunctionType
ALU = mybir.AluOpType

B, H, W, C = 32, 64, 64, 128
NCORES = 8
BPC = B // NCORES  # samples per core
HP, WP = H + 2, W + 2  # zero-padded
NPAD = HP * WP  # 4356
NPOS = H * W  # 4096
K = 4  # experts
NF = 128  # output filters
TAPS = 9
ROWS_PER_CHUNK = 8  # 8 image rows * 64 cols = 512 positions per PSUM chunk
NCHUNK = H // ROWS_PER_CHUNK


HALF = NPAD // 2  # split point for the pool reduce / input DMA halves


def _emit_chain(nc, b, sb, ps, xt_sb, consts, wk_sb, wm_sb, beta_sb):
    """Routing chain for sample b: pool -> attention -> mixed weights + beta."""
    red_sb, attk_sb, i4_sb, ones1_sb, ones4_sb, biasw_sb, c1_sb = consts

    # Global pool: sum over all padded positions (zeros don't change the
    # sum). Two halves so the first starts while the second half's DMA lands.
    pa = sb.tile([C, 1], F32, tag="poolh", name=f"poolA{b}")
    pb = sb.tile([C, 1], F32, tag="poolh", name=f"poolB{b}")
    nc.vector.tensor_reduce(pa[:], xt_sb[:, :HALF], axis=mybir.AxisListType.X,
                            op=ALU.add)
    nc.vector.tensor_reduce(pb[:], xt_sb[:, HALF:], axis=mybir.AxisListType.X,
                            op=ALU.add)
    pool_sb = sb.tile([C, 1], F32, tag="pool")
    nc.vector.tensor_add(pool_sb[:], pa[:], pb[:])

    # pool_red.T = (R/4096).T @ pool_sum  -> [4, 1]
    pr_ps = ps.tile([K, 1], F32, tag="tiny")
    nc.tensor.matmul(pr_ps[:], red_sb[:], pool_sb[:], start=True, stop=True)
    prelu_sb = sb.tile([K, 1], F32, tag="prelu")
    nc.scalar.activation(prelu_sb[:], pr_ps[:], AF.Relu)

    # logits.T = (A/30).T @ pool_red.T  -> [4, 1]
    lg_ps = ps.tile([K, 1], F32, tag="tiny")
    nc.tensor.matmul(lg_ps[:], attk_sb[:], prelu_sb[:], start=True, stop=True)
    e_sb = sb.tile([K, 1], F32, tag="esb")
    nc.scalar.activation(e_sb[:], lg_ps[:], AF.Exp)

    # softmax denominator and broadcast of its reciprocal to 4 partitions
    s_ps = ps.tile([1, 1], F32, tag="tiny")
    nc.tensor.matmul(s_ps[:], ones4_sb[:], e_sb[:], start=True, stop=True)
    rec_sb = sb.tile([1, 1], F32, tag="rec")
    nc.vector.reciprocal(rec_sb[:], s_ps[:])
    r4_ps = ps.tile([K, 1], F32, tag="tiny")
    nc.tensor.matmul(r4_ps[:], ones1_sb[:, 0:K], rec_sb[:], start=True,
                     stop=True)
    att_sb = sb.tile([K, 1], F32, tag="att")
    nc.vector.tensor_mul(att_sb[:], e_sb[:], r4_ps[:])

    # beta = biasw.T @ att + c1   (bias and BN folded; per-partition over F)
    bm_ps = ps.tile([NF, 1], F32, tag="tiny")
    nc.tensor.matmul(bm_ps[:], biasw_sb[:], att_sb[:], start=True, stop=True)
    nc.scalar.activation(beta_sb[:], bm_ps[:], AF.Identity, bias=c1_sb[:])

    # Broadcast att to all 128 partitions: att_row = att.T @ I4 -> [1, 4],
    # then att_bc = ones(1,128).T @ att_row -> [128, 4].
    ar_ps = ps.tile([1, K], F32, tag="tiny")
    nc.tensor.matmul(ar_ps[:], att_sb[:], i4_sb[:], start=True, stop=True)
    ar_sb = sb.tile([1, K], F32, tag="arow")
    nc.scalar.copy(ar_sb[:], ar_ps[:])
    ab_ps = ps.tile([C, K], F32, tag="tiny")
    nc.tensor.matmul(ab_ps[:], ones1_sb[:], ar_sb[:], start=True, stop=True)
    ab_sb = sb.tile([C, K], F32, tag="abc")
    nc.scalar.copy(ab_sb[:], ab_ps[:])

    # Mix expert bank: wm = sum_k att[k] * wk[k]   [128, 1152] -> f32r
    nc.vector.tensor_scalar_mul(wm_sb[:], wk_sb[0][:], ab_sb[:, 0:1])
    for k in range(1, K):
        nc.vector.scalar_tensor_tensor(wm_sb[:], wk_sb[k][:], ab_sb[:, k:k + 1],
                                       wm_sb[:], op0=ALU.mult, op1=ALU.add)


def _emit_conv_chunks(nc, b, convps, xt_sb, wm_sb, beta_sb, y_sb, y_dram,
                      t_lo, t_hi):
    """9-tap conv chunks [t_lo, t_hi) as shifted f32r matmuls + fused
    BN/bias/relu epilogue; output DMA'd out in half-sample pieces."""
    xv = xt_sb[:].rearrange("p (h w) -> p h w", w=WP)
    for t in range(t_lo, t_hi):
        pc = convps.tile([NF, ROWS_PER_CHUNK * W], F32, tag="conv")
        for tap in range(TAPS):
            dy, dx = tap // 3, tap % 3
            rhs = xv[:, ROWS_PER_CHUNK * t + dy:ROWS_PER_CHUNK * t + dy + ROWS_PER_CHUNK,
                     dx:dx + W]
            nc.tensor.matmul(pc[:], wm_sb[:, NF * tap:NF * (tap + 1)], rhs,
                             start=(tap == 0), stop=(tap == TAPS - 1))
        nc.scalar.activation(y_sb[:, 512 * t:512 * (t + 1)], pc[:], AF.Relu,
                             bias=beta_sb[:])
        if t == NCHUNK // 2 - 1:
            nc.sync.dma_start(y_dram[b][:, :NPOS // 2], y_sb[:, :NPOS // 2])
        elif t == NCHUNK - 1:
            nc.sync.dma_start(y_dram[b][:, NPOS // 2:], y_sb[:, NPOS // 2:])


def _build_program():
    nc = bacc.Bacc("TRN2", target_bir_lowering=False, debug=False,
                   num_devices=NCORES)
    xt = nc.dram_tensor("xt", [BPC, C, NPAD], F32R, kind="ExternalInput").ap()
    wk = nc.dram_tensor("wk", [K, C, TAPS * NF], F32R, kind="ExternalInput").ap()
    biasw = nc.dram_tensor("biasw", [K, NF], F32, kind="ExternalInput").ap()
    c1 = nc.dram_tensor("c1", [NF, 1], F32, kind="ExternalInput").ap()
    red = nc.dram_tensor("red", [C, K], F32, kind="ExternalInput").ap()
    attk = nc.dram_tensor("attk", [K, K], F32, kind="ExternalInput").ap()
    i4 = nc.dram_tensor("i4", [K, K], F32, kind="ExternalInput").ap()
    ones1 = nc.dram_tensor("ones1", [1, C], F32, kind="ExternalInput").ap()
    ones4 = nc.dram_tensor("ones4", [K, 1], F32, kind="ExternalInput").ap()
    y = nc.dram_tensor("y", [BPC, NF, NPOS], F32, kind="ExternalOutput").ap()

    with tile.TileContext(nc) as tc:
        with (
            tc.tile_pool(name="const", bufs=1) as cpool,
            tc.tile_pool(name="xt", bufs=BPC) as xpool,
            tc.tile_pool(name="wm", bufs=BPC) as wmpool,
            tc.tile_pool(name="work", bufs=4) as sb,
            tc.tile_pool(name="ystage", bufs=2) as ypool,
            tc.tile_pool(name="convps", bufs=4, space="PSUM") as convps,
            tc.tile_pool(name="tinyps", bufs=3, space="PSUM") as ps,
        ):
            # Constants
            red_sb = cpool.tile([C, K], F32)
            attk_sb = cpool.tile([K, K], F32)
            i4_sb = cpool.tile([K, K], F32)
            ones1_sb = cpool.tile([1, C], F32)
            ones4_sb = cpool.tile([K, 1], F32)
            biasw_sb = cpool.tile([K, NF], F32)
            c1_sb = cpool.tile([NF, 1], F32)
            nc.sync.dma_start(red_sb[:], red)
            nc.sync.dma_start(attk_sb[:], attk)
            nc.sync.dma_start(i4_sb[:], i4)
            nc.sync.dma_start(ones1_sb[:], ones1)
            nc.sync.dma_start(ones4_sb[:], ones4)
            nc.sync.dma_start(biasw_sb[:], biasw)
            nc.sync.dma_start(c1_sb[:], c1)
            wk_sb = []
            for k in range(K):
                wkt = cpool.tile([C, TAPS * NF], F32R, tag=f"wk{k}")
                nc.sync.dma_start(wkt[:], wk[k])
                wk_sb.append(wkt)
            consts = (red_sb, attk_sb, i4_sb, ones1_sb, ones4_sb, biasw_sb,
                      c1_sb)

            # Pre-load the ACT spline table set (relu+exp share one set).
            warm_sb = cpool.tile([1, 1], F32, tag="warm")
            nc.scalar.activation(warm_sb[:], ones4_sb[0:1, :], AF.Exp)

            # Staggered input loads: sample 0 gets HBM to itself (two halves
            # so the pool reduce starts on the first half early); sample b's
            # transfer is gated on sample b-1's finishing via explicit dep
            # edges, issued from the otherwise-idle GPSIMD queue.
            xt_sb = [xpool.tile([C, NPAD], F32R, tag="xt", name=f"xt{b}")
                     for b in range(BPC)]
            nc.sync.dma_start(xt_sb[0][:, :HALF], xt[0][:, :HALF])
            prev = nc.sync.dma_start(xt_sb[0][:, HALF:], xt[0][:, HALF:])
            for b in range(1, BPC):
                da = nc.gpsimd.dma_start(xt_sb[b][:, :HALF], xt[b][:, :HALF])
                db = nc.gpsimd.dma_start(xt_sb[b][:, HALF:], xt[b][:, HALF:])
                add_dep_helper(da.ins, prev.ins,
                               reason="stagger input DMA bandwidth")
                prev = db

            wm_sb = [wmpool.tile([C, TAPS * NF], F32R, tag="wm",
                                 name=f"wm{b}") for b in range(BPC)]
            beta_sb = [sb.tile([NF, 1], F32, tag="beta", name=f"beta{b}")
                       for b in range(BPC)]
            y_sb = [ypool.tile([NF, NPOS], F32, tag="ystage", name=f"yst{b}")
                    for b in range(BPC)]

            # PE warm-up: keep the array busy through the startup window so
            # HAM is at full clock when the conv stream begins. Garbage
            # results into a dedicated PSUM bank.
            warm_ps = ps.tile([NF, 512], F32, tag="warmps", bufs=1)
            wsrcA = wk_sb[0][:, 0:NF]
            wsrcB = wk_sb[1][:, 0:512]

            def pe_warm(n):
                for _ in range(n):
                    nc.tensor.matmul(warm_ps[:], wsrcA, wsrcB, start=True,
                                     stop=True)

            pe_warm(22)
            _emit_chain(nc, 0, sb, ps, xt_sb[0], consts, wk_sb, wm_sb[0],
                        beta_sb[0])
            pe_warm(20)
            # Interleave: conv_b chunks 0-2, then chain(b+1), then chunks 3-7.
            for b in range(BPC):
                _emit_conv_chunks(nc, b, convps, xt_sb[b], wm_sb[b],
                                  beta_sb[b], y_sb[b], y, 0, 3)
                if b + 1 < BPC:
                    _emit_chain(nc, b + 1, sb, ps, xt_sb[b + 1], consts,
                                wk_sb, wm_sb[b + 1], beta_sb[b + 1])
                _emit_conv_chunks(nc, b, convps, xt_sb[b], wm_sb[b],
                                  beta_sb[b], y_sb[b], y, 3, NCHUNK)

    nc.compile()
    return nc


_PROGRAM = None


def _get_program():
    global _PROGRAM
    if _PROGRAM is None:
        _PROGRAM = _build_program()
    return _PROGRAM


def _prepare_host_inputs(x, reduction_kernel, attention_kernel, conv_kernels,
                         bias, bn_scale, bn_bias, bn_mean, bn_var):
    f = np.float32
    # Channel-major zero-padded input: [B, C, 66, 66]
    xt = np.zeros((B, C, HP, WP), dtype=f)
    xt[:, :, 1:H + 1, 1:W + 1] = np.ascontiguousarray(x.transpose(0, 3, 1, 2))
    xt = xt.reshape(B, C, NPAD)

    inv = (bn_scale / np.sqrt(bn_var + np.float32(1e-5))).astype(f)
    # Expert bank [k, C, tap*F], BN scale folded into F.
    wkh = (conv_kernels.transpose(0, 3, 1, 2, 4) * inv).astype(f)
    wkh = np.ascontiguousarray(wkh.reshape(K, C, TAPS * NF))
    biasw = np.ascontiguousarray((bias * inv).astype(f))
    c1 = np.ascontiguousarray((bn_bias - bn_mean * inv).astype(f).reshape(NF, 1))
    red = np.ascontiguousarray((reduction_kernel / np.float32(NPOS)).astype(f))
    attk = np.ascontiguousarray((attention_kernel / np.float32(30.0)).astype(f))
    i4 = np.eye(K, dtype=f)
    ones1 = np.ones((1, C), dtype=f)
    ones4 = np.ones((K, 1), dtype=f)

    shared = {"wk": wkh, "biasw": biasw, "c1": c1, "red": red, "attk": attk,
              "i4": i4, "ones1": ones1, "ones4": ones4}
    in_maps = []
    for cix in range(NCORES):
        m = dict(shared)
        m["xt"] = np.ascontiguousarray(xt[cix * BPC:(cix + 1) * BPC])
        in_maps.append(m)
    return in_maps


def kernel(x, reduction_kernel, attention_kernel, conv_kernels, bias, bn_scale,
           bn_bias, bn_mean, bn_var, _trace=False):
    nc = _get_program()
    in_maps = _prepare_host_inputs(
        np.asarray(x, dtype=np.float32), np.asarray(reduction_kernel, np.float32),
        np.asarray(attention_kernel, np.float32),
        np.asarray(conv_kernels, np.float32), np.asarray(bias, np.float32),
        np.asarray(bn_scale, np.float32), np.asarray(bn_bias, np.float32),
        np.asarray(bn_mean, np.float32), np.asarray(bn_var, np.float32))
    if _trace:
        _ensure_ntff_hook()
    res = run_bass_kernel_spmd(nc, in_maps, core_ids=list(range(NCORES)),
                               trace=_trace)
    yt = np.concatenate([res.results[cix]["y"] for cix in range(NCORES)],
                        axis=0)  # [B, F, 4096]
    out = yt.reshape(B, NF, H, W).transpose(0, 2, 3, 1)
    out = np.ascontiguousarray(out, dtype=np.float32)
    if _trace:
        return out, res
    return out


# revision 22
# speedup vs baseline: 1.2554x; 1.1842x over previous
"""Self-contained Trainium2 kernel for nn_DynamicConv2D (moe_routing).

Contract: kernel(**inputs) takes FULL unsharded inputs (numpy), returns the
FULL output [32, 64, 64, 128] float32. Internally shards batch across 8
NeuronCores (4 samples each), runs a Bass/Tile kernel via
run_bass_kernel_spmd, and gathers.

Device-side work per sample:
  pool  = sum(x) over H,W            (scalar-engine Identity w/ accum_out;
                                      1/4096 folded into R on host)
  att   = softmax(relu(pool@R')@A')  (tiny PE matmuls + ACT relu/exp + DVE recip)
  wmix  = sum_k att[k] * bank[k]     (fused DVE scalar_tensor_tensor MACs, fp16)
  conv  = 9-tap shifted fp16 matmuls accumulated in PSUM, per 512-pos chunk
  out   = Relu(conv + beta)          (ACT epilogue, per-partition bias;
                                      BN scale folded into bank/bias on host)

Layout: x is host-transposed to channel-major [C, H, W], zero-padded to
[C, 66, 66], and cast to fp16 so all 9 conv taps are plain access-pattern
offsets; output is produced channel-major [F, H*W] f32 and host-transposed
back to NHWC. Expert bank is BN-folded, fp16, replicated per core.
"""

import sys

if "/opt/trn_rl_repo" not in sys.path:
    sys.path.insert(0, "/opt/trn_rl_repo")

import numpy as np

import concourse.bacc as bacc
import concourse.tile as tile
from concourse import mybir
from concourse.bass_utils import run_bass_kernel_spmd
from concourse.tile_rust import add_dep_helper


def _ensure_ntff_hook():
    """run_bass_kernel_spmd(trace=True) under axon needs antenv.axon_hooks,
    which this image's antenv package lacks. Register an equivalent module
    (ctypes into libaxon_pjrt.so) so profiled runs work."""
    try:
        from antenv import axon_hooks  # noqa: F401
        return
    except ImportError:
        pass
    import contextlib
    import ctypes
    import os
    import types

    so_path = os.environ.get("AXON_PJRT_SO", "/opt/axon/libaxon_pjrt.so")
    mod = types.ModuleType("antenv.axon_hooks")
    state = {"hook": None}

    def _make_hook():
        if not os.path.exists(so_path):
            return None
        lib = ctypes.CDLL(so_path)
        if not hasattr(lib, "axon_start_nrt_profile"):
            return None
        lib.axon_start_nrt_profile.argtypes = [
            ctypes.POINTER(ctypes.c_int64), ctypes.c_size_t]
        lib.axon_start_nrt_profile.restype = ctypes.c_int64
        lib.axon_stop_nrt_profile.argtypes = [ctypes.c_char_p]
        lib.axon_stop_nrt_profile.restype = ctypes.c_int64

        @contextlib.contextmanager
        def _hook(output_dir, device_ids):
            import jax
            jax.devices()
            if device_ids:
                ids = (ctypes.c_int64 * len(device_ids))(*device_ids)
                rc = lib.axon_start_nrt_profile(ids, len(device_ids))
            else:
                rc = lib.axon_start_nrt_profile(None, 0)
            if rc != 0:
                raise RuntimeError(f"axon_start_nrt_profile rc={rc}")
            try:
                yield
            finally:
                n = lib.axon_stop_nrt_profile(str(output_dir).encode())
                if n < 0:
                    raise RuntimeError(f"axon_stop_nrt_profile rc={n}")

        return _hook

    def get_axon_ntff_profile_hook():
        if state["hook"] is None:
            state["hook"] = _make_hook()
        return state["hook"]

    def set_axon_ntff_profile_hook(hook):
        state["hook"] = hook

    mod.get_axon_ntff_profile_hook = get_axon_ntff_profile_hook
    mod.set_axon_ntff_profile_hook = set_axon_ntff_profile_hook
    sys.modules["antenv.axon_hooks"] = mod
    try:
        import antenv
        antenv.axon_hooks = mod
    except ImportError:
        pass


F32 = mybir.dt.float32
F16 = mybir.dt.float16
AF = mybir.ActivationFunctionType
ALU = mybir.AluOpType

B, H, W, C = 32, 64, 64, 128
NCORES = 8
BPC = B // NCORES  # samples per core
HP, WP = H + 2, W + 2  # zero-padded
NPAD = HP * WP  # 4356
NPOS = H * W  # 4096
K = 4  # experts
NF = 128  # output filters
TAPS = 9
ROWS_PER_CHUNK = 8  # 8 image rows * 64 cols = 512 positions per PSUM chunk
NCHUNK = H // ROWS_PER_CHUNK
HALF = NPAD // 2
# packed-constant column layout (one [128, CST_COLS] f32 tensor, one DMA)
CST_COLS = 270

# tunables
WARM1, WARM2 = 10, 12  # PE warm-up matmuls before/after chain-0 tiny MMs


class _Consts:
    """AP views into the packed constant SBUF tile."""

    def __init__(self, cst):
        self.red = cst[:, 0:4]        # reduction_kernel / 4096   [128, 4]
        self.c1 = cst[:, 4:5]         # bn_bias - bn_mean*inv     [128, 1]
        self.attk = cst[0:4, 5:9]     # attention_kernel / 30     [4, 4]
        self.i4 = cst[0:4, 9:13]      # identity                  [4, 4]
        self.ones4 = cst[0:4, 13:14]  # ones                      [4, 1]
        self.biasw = cst[0:4, 14:142]  # bias * inv               [4, 128]
        self.ones1 = cst[0:1, 142:270]  # ones                    [1, 128]


def _pack_consts(red, c1, attk, i4, ones4, biasw, ones1):
    cst = np.zeros((128, CST_COLS), dtype=np.float32)
    cst[:, 0:4] = red
    cst[:, 4] = c1
    cst[0:4, 5:9] = attk
    cst[0:4, 9:13] = i4
    cst[0:4, 13] = ones4
    cst[0:4, 14:142] = biasw
    cst[0, 142:270] = ones1
    return cst


def _emit_chain(nc, b, sb, ps, xt_sb, cc, wk_sb, wm_sb, beta_sb, trash):
    """Routing chain for sample b: pool -> attention -> mixed weights + beta.

    Returns (pa_inst, pb_inst) — the two pool half-reduces, used to gate the
    next sample's input DMA on this sample's input being fully resident."""
    # Global pool via scalar-engine Identity with running-sum accumulator
    # (zeros in the padding don't change the sum). Two halves so the first
    # starts while the second half's DMA is still landing.
    pa = sb.tile([C, 1], F32, tag="poolh", name=f"poolA{b}")
    pb = sb.tile([C, 1], F32, tag="poolh", name=f"poolB{b}")
    ia = nc.scalar.activation(trash[:], xt_sb[:, :HALF], AF.Identity,
                              accum_out=pa[:])
    ib = nc.scalar.activation(trash[:], xt_sb[:, HALF:], AF.Identity,
                              accum_out=pb[:])

    # pool_red.T = (R/4096).T @ (pa + pb) via two accumulating matmuls
    pr_ps = ps.tile([K, 1], F32, tag="tiny")
    nc.tensor.matmul(pr_ps[:], cc.red, pa[:], start=True, stop=False)
    nc.tensor.matmul(pr_ps[:], cc.red, pb[:], start=False, stop=True)
    prelu_sb = sb.tile([K, 1], F32, tag="prelu")
    nc.scalar.activation(prelu_sb[:], pr_ps[:], AF.Relu)

    # logits.T = (A/30).T @ pool_red.T  -> [4, 1]
    lg_ps = ps.tile([K, 1], F32, tag="tiny")
    nc.tensor.matmul(lg_ps[:], cc.attk, prelu_sb[:], start=True, stop=True)
    e_sb = sb.tile([K, 1], F32, tag="esb")
    nc.scalar.activation(e_sb[:], lg_ps[:], AF.Exp)

    # softmax denominator and broadcast of its reciprocal to 4 partitions
    s_ps = ps.tile([1, 1], F32, tag="tiny")
    nc.tensor.matmul(s_ps[:], cc.ones4, e_sb[:], start=True, stop=True)
    rec_sb = sb.tile([1, 1], F32, tag="rec")
    nc.vector.reciprocal(rec_sb[:], s_ps[:])
    r4_ps = ps.tile([K, 1], F32, tag="tiny")
    nc.tensor.matmul(r4_ps[:], cc.ones1[:, 0:K], rec_sb[:], start=True,
                     stop=True)
    att_sb = sb.tile([K, 1], F32, tag="att")
    nc.vector.tensor_mul(att_sb[:], e_sb[:], r4_ps[:])

    # beta = biasw.T @ att + c1   (bias and BN folded; per-partition over F)
    bm_ps = ps.tile([NF, 1], F32, tag="tiny")
    nc.tensor.matmul(bm_ps[:], cc.biasw, att_sb[:], start=True, stop=True)
    nc.scalar.activation(beta_sb[:], bm_ps[:], AF.Identity, bias=cc.c1)

    # Broadcast att to all 128 partitions: att_row = att.T @ I4 -> [1, 4],
    # then att_bc = ones(1,128).T @ att_row -> [128, 4].
    ar_ps = ps.tile([1, K], F32, tag="tiny")
    nc.tensor.matmul(ar_ps[:], att_sb[:], cc.i4, start=True, stop=True)
    ar_sb = sb.tile([1, K], F32, tag="arow")
    nc.scalar.copy(ar_sb[:], ar_ps[:])
    ab_ps = ps.tile([C, K], F32, tag="tiny")
    nc.tensor.matmul(ab_ps[:], cc.ones1, ar_sb[:], start=True, stop=True)
    ab_sb = sb.tile([C, K], F32, tag="abc")
    nc.scalar.copy(ab_sb[:], ab_ps[:])

    # Mix expert bank: wm = sum_k att[k] * wk[k]   [128, 1152] fp16
    nc.vector.tensor_scalar_mul(wm_sb[:], wk_sb(0), ab_sb[:, 0:1])
    for k in range(1, K):
        nc.vector.scalar_tensor_tensor(wm_sb[:], wk_sb(k), ab_sb[:, k:k + 1],
                                       wm_sb[:], op0=ALU.mult, op1=ALU.add)
    return ia, ib


def _emit_conv_chunks(nc, b, convps, xt_sb, wm_sb, beta_sb, y_sb, y_dram,
                      t_lo, t_hi):
    """9-tap conv chunks [t_lo, t_hi) as shifted fp16 matmuls + fused
    BN/bias/relu epilogue; output DMA'd out in pieces to shrink the tail."""
    xv = xt_sb.rearrange("p (h w) -> p h w", w=WP)
    for t in range(t_lo, t_hi):
        pc = convps.tile([NF, ROWS_PER_CHUNK * W], F32, tag="conv")
        for tap in range(TAPS):
            dy, dx = tap // 3, tap % 3
            r0 = ROWS_PER_CHUNK * t + dy
            rhs = xv[:, r0:r0 + ROWS_PER_CHUNK, dx:dx + W]
            nc.tensor.matmul(pc[:], wm_sb[:, NF * tap:NF * (tap + 1)], rhs,
                             start=(tap == 0), stop=(tap == TAPS - 1))
        nc.scalar.activation(y_sb[:, 512 * t:512 * (t + 1)], pc[:], AF.Relu,
                             bias=beta_sb[:])
        if t == 3:
            nc.sync.dma_start(y_dram[b][:, :2048], y_sb[:, :2048])
        elif t == 5:
            nc.sync.dma_start(y_dram[b][:, 2048:3072], y_sb[:, 2048:3072])
        elif t == 7:
            nc.sync.dma_start(y_dram[b][:, 3072:], y_sb[:, 3072:])


def _build_program():
    nc = bacc.Bacc("TRN2", target_bir_lowering=False, debug=False,
                   num_devices=NCORES)
    xt = nc.dram_tensor("xt", [BPC, C, NPAD], F16, kind="ExternalInput").ap()
    wk = nc.dram_tensor("wk", [C, K * TAPS * NF], F16,
                        kind="ExternalInput").ap()
    cstd = nc.dram_tensor("cst", [128, CST_COLS], F32,
                          kind="ExternalInput").ap()
    y = nc.dram_tensor("y", [BPC, NF, NPOS], F32, kind="ExternalOutput").ap()

    with tile.TileContext(nc) as tc:
        with (
            tc.tile_pool(name="const", bufs=1) as cpool,
            tc.tile_pool(name="xt", bufs=BPC) as xpool,
            tc.tile_pool(name="wm", bufs=BPC) as wmpool,
            tc.tile_pool(name="work", bufs=4) as sb,
            tc.tile_pool(name="ystage", bufs=2) as ypool,
            tc.tile_pool(name="convps", bufs=4, space="PSUM") as convps,
            tc.tile_pool(name="tinyps", bufs=3, space="PSUM") as ps,
        ):
            xt_sb = [xpool.tile([C, NPAD], F16, tag="xt", name=f"xt{b}")
                     for b in range(BPC)]
            # sample 0 first at full bandwidth (two halves so the pool can
            # start on the first half)
            nc.sync.dma_start(xt_sb[0][:, :HALF], xt[0][:, :HALF])
            nc.sync.dma_start(xt_sb[0][:, HALF:], xt[0][:, HALF:])
            cst = cpool.tile([128, CST_COLS], F32)
            nc.sync.dma_start(cst[:], cstd)
            wk_all = cpool.tile([C, K * TAPS * NF], F16)
            nc.sync.dma_start(wk_all[:], wk)
            cc = _Consts(cst[:])

            def wk_sb(k):
                return wk_all[:, k * TAPS * NF:(k + 1) * TAPS * NF]

            # samples 1-3: issued from the idle GPSIMD queue; transfer start
            # gated (via dep edges added below) on the previous sample's
            # pool reduce, i.e. on its input being fully resident.
            xt_dma = [None] * BPC
            for b in range(1, BPC):
                da = nc.gpsimd.dma_start(xt_sb[b][:, :HALF], xt[b][:, :HALF])
                db = nc.gpsimd.dma_start(xt_sb[b][:, HALF:], xt[b][:, HALF:])
                xt_dma[b] = (da, db)

            # Pre-load the ACT spline table set (relu+exp share one set).
            warm_sb = cpool.tile([1, 1], F32, tag="warm")
            nc.scalar.activation(warm_sb[:], cc.ones4[0:1, :], AF.Exp)

            trash = cpool.tile([C, HALF], F16, tag="trash")

            wm_sb = [wmpool.tile([C, TAPS * NF], F16, tag="wm",
                                 name=f"wm{b}") for b in range(BPC)]
            beta_sb = [sb.tile([NF, 1], F32, tag="beta", name=f"beta{b}")
                       for b in range(BPC)]
            y_sb = [ypool.tile([NF, NPOS], F32, tag="ystage", name=f"yst{b}")
                    for b in range(BPC)]

            # PE warm-up: keep the array busy through the startup window so
            # HAM is at full clock when the conv stream begins.
            warm_ps = ps.tile([NF, 512], F32, tag="warmps", bufs=1)

            def pe_warm(n):
                for _ in range(n):
                    nc.tensor.matmul(warm_ps[:], wk_all[:, 0:NF],
                                     wk_all[:, 0:512], start=True, stop=True)

            pe_warm(WARM1)
            pool_insts = [None] * BPC
            pool_insts[0] = _emit_chain(nc, 0, sb, ps, xt_sb[0][:], cc, wk_sb,
                                        wm_sb[0], beta_sb[0], trash)
            pe_warm(WARM2)
            # Interleave: conv_b chunks 0-2, then chain(b+1), then chunks 3-7.
            for b in range(BPC):
                _emit_conv_chunks(nc, b, convps, xt_sb[b][:], wm_sb[b],
                                  beta_sb[b], y_sb[b], y, 0, 3)
                if b + 1 < BPC:
                    pool_insts[b + 1] = _emit_chain(
                        nc, b + 1, sb, ps, xt_sb[b + 1][:], cc, wk_sb,
                        wm_sb[b + 1], beta_sb[b + 1], trash)
                _emit_conv_chunks(nc, b, convps, xt_sb[b][:], wm_sb[b],
                                  beta_sb[b], y_sb[b], y, 3, NCHUNK)

            # DMA staggering: sample b's transfer starts only once sample
            # b-1's input is fully resident (its pool half-reduces ran).
            for b in range(1, BPC):
                da, db = xt_dma[b]
                prev_pa, prev_pb = pool_insts[b - 1]
                add_dep_helper(da.ins, prev_pa.ins,
                               reason="stagger input DMA bandwidth")
                add_dep_helper(db.ins, prev_pb.ins,
                               reason="stagger input DMA bandwidth")

    nc.compile()
    return nc


_PROGRAM = None


def _get_program():
    global _PROGRAM
    if _PROGRAM is None:
        _PROGRAM = _build_program()
    return _PROGRAM


def _prepare_host_inputs(x, reduction_kernel, attention_kernel, conv_kernels,
                         bias, bn_scale, bn_bias, bn_mean, bn_var):
    f = np.float32
    # Channel-major zero-padded fp16 input: [B, C, 66, 66]
    xt = np.zeros((B, C, HP, WP), dtype=np.float16)
    xt[:, :, 1:H + 1, 1:W + 1] = x.transpose(0, 3, 1, 2)
    xt = xt.reshape(B, C, NPAD)

    inv = (bn_scale / np.sqrt(bn_var + np.float32(1e-5))).astype(f)
    # Expert bank [C, k*tap*F] fp16, BN scale folded into F.
    wkh = (conv_kernels.transpose(0, 3, 1, 2, 4) * inv).astype(f)
    wkh = wkh.reshape(K, C, TAPS * NF).transpose(1, 0, 2).reshape(
        C, K * TAPS * NF)
    wkh = np.ascontiguousarray(wkh, dtype=np.float16)

    cst = _pack_consts(
        red=(reduction_kernel / np.float32(NPOS)).astype(f),
        c1=(bn_bias - bn_mean * inv).astype(f),
        attk=(attention_kernel / np.float32(30.0)).astype(f),
        i4=np.eye(K, dtype=f),
        ones4=np.ones(K, dtype=f),
        biasw=(bias * inv).astype(f),
        ones1=np.ones(C, dtype=f),
    )

    in_maps = []
    for cix in range(NCORES):
        in_maps.append({
            "xt": np.ascontiguousarray(xt[cix * BPC:(cix + 1) * BPC]),
            "wk": wkh,
            "cst": cst,
        })
    return in_maps


def kernel(x, reduction_kernel, attention_kernel, conv_kernels, bias, bn_scale,
           bn_bias, bn_mean, bn_var, _trace=False):
    nc = _get_program()
    in_maps = _prepare_host_inputs(
        np.asarray(x, dtype=np.float32), np.asarray(reduction_kernel, np.float32),
        np.asarray(attention_kernel, np.float32),
        np.asarray(conv_kernels, np.float32), np.asarray(bias, np.float32),
        np.asarray(bn_scale, np.float32), np.asarray(bn_bias, np.float32),
        np.asarray(bn_mean, np.float32), np.asarray(bn_var, np.float32))
    if _trace:
        _ensure_ntff_hook()
    res = run_bass_kernel_spmd(nc, in_maps, core_ids=list(range(NCORES)),
                               trace=_trace)
    yt = np.concatenate([res.results[cix]["y"] for cix in range(NCORES)],
                        axis=0)  # [B, F, 4096]
    out = yt.reshape(B, NF, H, W).transpose(0, 2, 3, 1)
    out = np.ascontiguousarray(out, dtype=np.float32)
    if _trace:
        return out, res
    return out
